# revision 26
# baseline (speedup 1.0000x reference)
"""Bass/Tile kernel for KernelAttention (linear attention with exp random features).

Per batch b:  wk = exp(K @ W); kv = wk.T @ V; wq = exp(Q @ W); out = wq @ kv.

Sharding: 8 cores = 4 batches x 2 n-halves of Q. K-side computed redundantly
per core pair.

Per-core design:
- Host pre-transposes/converts inputs: q8 = fp8(Q^T * sqrt(A8)) in DoubleRow
  layout [32, 2, 16384]; k_t bf16 [64, 4096]; v bf16 [128, 32, 64] (scaled);
  w bf16; w8 = fp8(W * sqrt(A8)) [32, 2, 256].
- wq matmul: fp8 DoubleRow, stationary W-half, moving q8 -> psum = A8*(q.w),
  in two phases (all r-half-0 blocks, then all r-half-1) so the stationary
  loads only twice.
- exp: per block-pair either exact (scalar activation Exp -> fp8) or a
  "pair Schraudolph": u8 bits v1 = round(psum + B1), v2 = round(psum + B1+4);
  the out-matmul sums the two phase-shifted approximations via two kv
  stationaries (cancels the exp2 linear-interp sawtooth).
- out matmul: fp8 DoubleRow, stationary kv8 variants, out^T [64, 512] per
  block; even/odd blocks pack into one [128, 512] psum tile.
- K-side: bf16 wk matmul (k_t chunks stationary); exp split scalar-exact /
  DVE bf16-pair-schraudolph; kv^T accumulation; PE transpose; fp8 convert.

Shapes hardcoded: B=4, N=4096, H=8, D=64, R=256.
"""

import math
import sys

sys.path.insert(0, "/opt/trn_rl_repo")

from contextlib import ExitStack

import ml_dtypes
import numpy as np

import concourse.bacc as bacc
import concourse.mybir as mybir
import concourse.tile as tile
from concourse import bass_utils

B, N, H, D, R = 4, 4096, 8, 64, 256
NCORES = 8
NH = (N // 2) * H          # 16384 q-rows per core
NBLK = NH // 512           # 32 out blocks of 512 rows
NPAIR = NBLK // 2          # 16 block pairs
KC = N // 128              # 32 k-chunks

FP32 = mybir.dt.float32
BF16 = mybir.dt.bfloat16
FP8 = mybir.dt.float8e4
U8 = mybir.dt.uint8
I16 = mybir.dt.int16
EXP = mybir.ActivationFunctionType.Exp
COPY = mybir.ActivationFunctionType.Copy
ADD = mybir.AluOpType.add
MULT = mybir.AluOpType.mult
DR = mybir.MatmulPerfMode.DoubleRow

A8 = 8.0 / math.log(2.0)            # fp8 bits per e-fold
SA8 = math.sqrt(A8)
C8P = -1.65                          # fp8 pair-schraudolph offset (round-nearest)
B8_1 = 56.0 + C8P
B8_2 = B8_1 + 4.0
WA8, WB8 = 0.555, 0.39244            # fp8 pair combination weights
A16 = 128.0 / math.log(2.0)          # bf16 bits per e-fold
B16P = 16256.0 - 6.9                 # bf16 pair offset
PAIR16_SCALE = 1.0 + 2.0 ** -0.5     # wk pair outputs (1+1/sqrt2)*exp
WK_BIAS = math.log(PAIR16_SCALE)     # scalar wk tiles match via exp bias
VSCALE = 0.25 / PAIR16_SCALE         # host folds into V
OSCALE = 0.25                        # kv8 tiles fold 1/OSCALE

# per-pair conv type: 'E' exact (scalar), 'R' pair-schraudolph
# (v1 = DVE convert from psum; v2 = v1 + 4 via uint8 accumulate-DMA)
PAIR_TYPE = ['E'] * 3 + ['R'] * 10 + ['E'] * 3   # R contiguous mid, E at ends
R_ENGINES = ['D']                    # engine cycle for R op1 units
WK_ASSIGN = ['S'] * 8                # wk conv tiles
OC_ENGINES = ['D', 'S'] * 8


def _build_program():
    nc = bacc.Bacc(
        "TRN2",
        target_bir_lowering=False,
        debug=False,
        enable_asserts=False,
        num_devices=NCORES,
    )
    qt = nc.dram_tensor("qt", [64, NH], FP8, kind="ExternalInput").ap()
    kt = nc.dram_tensor("kt", [64, N], FP8, kind="ExternalInput").ap()
    w8 = nc.dram_tensor("w8", [64, R], FP8, kind="ExternalInput").ap()
    v = nc.dram_tensor("v", [128, KC, D], BF16, kind="ExternalInput").ap()
    w = nc.dram_tensor("w", [64, R], BF16, kind="ExternalInput").ap()
    ident = nc.dram_tensor("ident", [128, 128], BF16, kind="ExternalInput").ap()
    o = nc.dram_tensor("o", [NPAIR, 128, 512], BF16, kind="ExternalOutput").ap()

    def eng(c):
        return {'S': nc.scalar, 'D': nc.vector, 'P': nc.gpsimd}[c]

    with tile.TileContext(nc) as tc, ExitStack() as ctx:
        consts = ctx.enter_context(tc.tile_pool(name="consts", bufs=1))
        id_sb = consts.tile([128, 128], BF16, tag="id")
        w_sb = consts.tile([64, R], BF16, tag="w")
        w8_sb = consts.tile([64, R], FP8, tag="w8")
        kt_sb = consts.tile([64, N], FP8, tag="kt")
        v_sb = consts.tile([128, KC, D], BF16, tag="v")
        kvt_sb = consts.tile([64, R], BF16, tag="kvt")
        # zero-padded out-matmul stationaries: lo fills out partitions 0:64
        # (even block), hi fills 64:128 (odd block, accumulated)
        kv8s = {}
        for nm in ("e_lo", "e_hi", "a_lo", "a_hi", "b_lo", "b_hi"):
            kv8s[nm] = consts.tile([128, 2, 128], FP8, tag=f"kv8{nm}",
                                   name=f"kv8{nm}")
            nc.vector.memset(kv8s[nm][:], 0.0)
        # wqe storage: [128, blk, rblock, 512] fp8 (v2 only written for R pairs)
        wqe1 = consts.tile([128, NBLK, 2, 512], FP8, tag="wqe1")
        wqe2 = consts.tile([128, NBLK, 2, 512], FP8, tag="wqe2")
        dummy = consts.tile([128, 8], FP32, tag="dummy")
        dummy2 = consts.tile([128, 8], FP32, tag="dummy2")
        wkbias = consts.tile([128, 1], FP32, tag="wkbias")
        nc.vector.memset(wkbias[:], WK_BIAS)
        four_sb = consts.tile([128, 1024], U8, tag="four")
        nc.vector.memset(four_sb[:], 4)

        qtpool = ctx.enter_context(tc.tile_pool(name="qtp", bufs=4))
        wkepool = ctx.enter_context(tc.tile_pool(name="wkep", bufs=8))
        i16pool = ctx.enter_context(tc.tile_pool(name="i16p", bufs=2))
        osbpool = ctx.enter_context(tc.tile_pool(name="osbp", bufs=3))
        # PSUM: mmps [128,2,512] fp32 x2 = 4 banks; ops [128,512] x2 = 2;
        # kvps 1; trps 1 -> 8 banks
        mmps = ctx.enter_context(tc.tile_pool(name="mmps", bufs=2, space="PSUM"))
        ops = ctx.enter_context(tc.tile_pool(name="ops", bufs=2, space="PSUM"))
        kvpsp = ctx.enter_context(tc.tile_pool(name="kvpsp", bufs=1, space="PSUM"))
        trpsp = ctx.enter_context(tc.tile_pool(name="trpsp", bufs=1, space="PSUM"))

        nc.vector.memset(dummy[:], 0.0)
        # ---- input DMAs on 3 rings ----
        # sync ring: kt, v (K-side feed), then v2 presets + output tiles
        # scalar ring: qt chunks 0-1;  gpsimd ring: w, ident, qt chunks 2-3
        for t in range(2):
            nc.sync.dma_start(
                kt_sb[:, 2048 * t : 2048 * (t + 1)],
                kt[:, 2048 * t : 2048 * (t + 1)],
            )
        for t in range(2):
            nc.sync.dma_start(
                v_sb[:, 16 * t : 16 * (t + 1), :], v[:, 16 * t : 16 * (t + 1), :]
            )
        qtc = []
        for t in range(4):
            qtt = qtpool.tile([64, 4096], FP8, tag="qt", name=f"qt_{t}")
            qtc.append(qtt)
        nc.scalar.dma_start(w8_sb[:], w8)
        nc.scalar.dma_start(qtc[0][:], qt[:, 0:4096])
        nc.scalar.dma_start(qtc[1][:], qt[:, 4096:8192])
        nc.scalar.dma_start(w_sb[:], w)
        nc.scalar.dma_start(id_sb[:], ident)
        nc.gpsimd.dma_start(qtc[2][:], qt[:, 8192:12288])
        nc.gpsimd.dma_start(qtc[3][:], qt[:, 12288:16384])
        # warm the scalar-engine exp table (after DMA issues, before convs)
        nc.scalar.activation(dummy2[:], dummy[:], EXP)
        # preset v2 slots of R pairs with 4s (bits offset for the pair trick)
        for p in range(NPAIR):
            if PAIR_TYPE[p] == 'R':
                for h in range(2):
                    nc.sync.dma_start(
                        wqe2[:, 2 * p : 2 * p + 2, h, :].bitcast(U8), four_sb[:]
                    )

        # ---- K-side: wk = exp(K @ W), tiles of 4 chunks in [128,2,512] ----
        wkes = []
        for t in range(8):
            wkps = mmps.tile([128, 2, 512], FP32, tag="mm", name=f"wkps{t}")
            for j in range(4):
                c = 4 * t + j
                nc.tensor.matmul(
                    wkps[:, j // 2, 256 * (j % 2) : 256 * (j % 2 + 1)],
                    kt_sb[:, 128 * c : 128 * (c + 1)],
                    w8_sb[:],
                )
            wke = wkepool.tile([128, 2, 512], BF16, tag="wke", name=f"wke{t}")
            if WK_ASSIGN[t] == 'S':
                nc.scalar.activation(wke[:], wkps[:], EXP, scale=1.0 / A8,
                                     bias=wkbias[:])
            else:
                i1 = i16pool.tile([128, 2, 512], I16, tag="i16a", name=f"i16a{t}")
                i2 = i16pool.tile([128, 2, 512], I16, tag="i16b", name=f"i16b{t}")
                nc.vector.tensor_scalar(i1[:], wkps[:], A16, B16P, MULT, ADD)
                nc.vector.tensor_scalar(i2[:], i1[:], -64, None, ADD)
                nc.vector.tensor_tensor(
                    wke[:], i1[:].bitcast(BF16), i2[:].bitcast(BF16), ADD
                )
            wkes.append(wke)

        # ---- Q-side phase h: wq psum = A8*(q.w) for r-half h ----
        r_rr = [0]

        def conv_pair(p, h, wqps):
            dst1 = wqe1[:, 2 * p : 2 * p + 2, h, :]
            if PAIR_TYPE[p] == 'E':
                nc.scalar.activation(dst1, wqps[:], EXP, scale=1.0 / A8)
            else:
                # split the convert across scalar and DVE halves (halves the
                # pipeline latency); v2 batches via accumulate-DMA later
                u1 = dst1.bitcast(U8)
                nc.scalar.activation(u1[:, :, 0:192], wqps[:, :, 0:192],
                                     COPY, bias=B8_1)
                nc.vector.tensor_scalar(u1[:, :, 192:512], wqps[:, :, 192:512],
                                        B8_1, None, ADD)
            if p in (6, 9, 12):
                lo = {6: 3, 9: 7, 12: 10}[p]
                nc.gpsimd.dma_start(
                    wqe2[:, 2 * lo : 2 * p + 2, h, :].bitcast(U8),
                    wqe1[:, 2 * lo : 2 * p + 2, h, :].bitcast(U8),
                    accum_op=ADD,
                )

        def wq_mm(p, h):
            wqps = mmps.tile([128, 2, 512], FP32, tag="mm", name=f"wqps{h}_{p}")
            for j in range(2):
                blk = 2 * p + j
                ch = qtc[blk // 8]
                col = (blk % 8) * 512
                nc.tensor.matmul(
                    wqps[:, j, :],
                    w8_sb[:, 128 * h : 128 * (h + 1)],
                    ch[:, col : col + 512],
                )
            return wqps

        for p in range(NPAIR):
            wqps = wq_mm(p, 0)
            conv_pair(p, 0, wqps)

        # ---- kv^T accumulation over all 32 chunks ----
        kvps = kvpsp.tile([64, R], FP32, tag="kvps")
        for c in range(KC):
            nc.tensor.matmul(
                kvps[:],
                v_sb[:, c, :],
                wkes[c // 4][:, (c % 4) // 2, 256 * (c % 2) : 256 * (c % 2 + 1)],
                start=(c == 0),
                stop=(c == KC - 1),
            )
        nc.scalar.activation(kvt_sb[:], kvps[:], COPY)
        # transpose kv^T -> kv [256, 64] (bf16 psum), then fp8 converts
        trps = trpsp.tile([128, 2, D], BF16, tag="trps")
        for j in range(2):
            nc.tensor.matmul(
                trps[:, j, :],
                kvt_sb[:, 128 * j : 128 * (j + 1)],
                id_sb[0:64, 0:64],
                is_transpose=True,
            )
        # kv8 tiles hold 0.25*kv_true (fp8e4 max is 240; kv peaks ~±300);
        # the final output copy scales by 4.
        for nm, sc in (("e", 1.0), ("a", WA8), ("b", WB8)):
            for half, off in (("lo", 0), ("hi", 64)):
                nc.vector.tensor_scalar(
                    kv8s[f"{nm}_{half}"][:, :, off : off + 64],
                    trps[:],
                    sc,
                    None,
                    MULT,
                )

        # ---- Q-side phase 1 interleaved with out matmuls ----
        def block_mms(p, j):
            """(stationary, rhs) list for block 2p+j of pair p."""
            blk = 2 * p + j
            half = 'lo' if j == 0 else 'hi'
            if PAIR_TYPE[p] == 'E':
                return [(kv8s[f"e_{half}"], wqe1[:, blk, :, :])]
            return [(kv8s[f"a_{half}"], wqe1[:, blk, :, :]),
                    (kv8s[f"b_{half}"], wqe2[:, blk, :, :])]

        def finish_pair(p, opst):
            o_sb = osbpool.tile([128, 512], BF16, tag="osb", name=f"osb{p}")
            if OC_ENGINES[p] == 'S':
                nc.scalar.activation(o_sb[:], opst[:], COPY, scale=4.0)
            else:
                nc.vector.tensor_scalar(o_sb[:], opst[:], 4.0, None, MULT)
            nc.sync.dma_start(o[p], o_sb[:])

        def emit_duo(pa, pb):
            """Out-matmuls for two pairs, interleaved so each stationary is
            used twice in a row (hides the DoubleRow weight loads)."""
            tiles = {pa: ops.tile([128, 512], FP32, tag="ops", name=f"ops{pa}"),
                     pb: ops.tile([128, 512], FP32, tag="ops", name=f"ops{pb}")}
            seqs = {p: block_mms(p, 0) + block_mms(p, 1) for p in (pa, pb)}
            nmax = max(len(seqs[pa]), len(seqs[pb]))
            for i in range(nmax):
                for p in (pa, pb):
                    if i < len(seqs[p]):
                        st, rhs = seqs[p][i]
                        nc.tensor.matmul(
                            tiles[p][:], st[:], rhs, perf_mode=DR,
                            start=(i == 0), stop=(i == len(seqs[p]) - 1),
                            skip_group_check=True,
                        )
            for p in (pa, pb):
                finish_pair(p, tiles[p])

        h1tiles = {}
        for g in range(8):  # groups of 2 pairs
            for p in range(2 * g, 2 * g + 2):
                h1tiles[p] = wq_mm(p, 1)
            for p in range(2 * g, 2 * g + 2):
                conv_pair(p, 1, h1tiles[p])
            if g >= 2:
                emit_duo(2 * (g - 2), 2 * (g - 2) + 1)
        emit_duo(12, 13)
        emit_duo(14, 15)

    nc.compile()
    return nc


_NC = None


def _get_nc():
    global _NC
    if _NC is None:
        _NC = _build_program()
    return _NC


def kernel(Q, K, V, W):
    nc = _get_nc()
    ident = np.eye(128, dtype=ml_dtypes.bfloat16)
    fp8np = mybir.dt.np(FP8)
    w_bf = np.ascontiguousarray(W).astype(ml_dtypes.bfloat16)
    w8_np = np.ascontiguousarray(W * SA8).astype(fp8np)
    in_maps = []
    for c in range(NCORES):
        b, half = c // 2, c % 2
        qs = Q[b, half * (N // 2) : (half + 1) * (N // 2)].reshape(NH, D)
        qt_np = np.ascontiguousarray(qs.T * SA8).astype(fp8np)
        kt_np = np.ascontiguousarray(K[b].T * SA8).astype(fp8np)
        v_np = np.ascontiguousarray(
            (V[b] * VSCALE).reshape(KC, 128, D).transpose(1, 0, 2)
        ).astype(ml_dtypes.bfloat16)
        in_maps.append({
            "qt": qt_np,
            "kt": kt_np,
            "v": v_np,
            "w": w_bf,
            "w8": w8_np,
            "ident": ident,
        })
    global _LAST_IN_MAPS
    _LAST_IN_MAPS = in_maps
    res = bass_utils.run_bass_kernel_spmd(nc, in_maps, core_ids=list(range(NCORES)))
    out = np.empty((B, N, H, D), np.float32)
    out_t = np.empty((D, NH), np.float32)
    for c in range(NCORES):
        b, half = c // 2, c % 2
        ob = res.results[c]["o"].astype(np.float32)   # [16, 128, 512]
        for p in range(NPAIR):
            out_t[:, 1024 * p : 1024 * p + 512] = ob[p, 0:64]
            out_t[:, 1024 * p + 512 : 1024 * (p + 1)] = ob[p, 64:128]
        out[b, half * (N // 2) : (half + 1) * (N // 2)] = out_t.T.reshape(
            N // 2, H, D
        )
    return out


# revision 27
# speedup vs baseline: 1.2272x; 1.2272x over previous
"""Bass/Tile kernel for KernelAttention (linear attention with exp random features).

Per batch b:  wk = exp(K @ W); kv = wk.T @ V; wq = exp(Q @ W); out = wq @ kv.

Sharding: 8 cores = 4 batches x 2 n-halves of Q. K-side computed redundantly
per core pair.

Per-core design:
- Host pre-transposes/converts inputs: q8 = fp8(Q^T * sqrt(A8)) in DoubleRow
  layout [32, 2, 16384]; k_t bf16 [64, 4096]; v bf16 [128, 32, 64] (scaled);
  w bf16; w8 = fp8(W * sqrt(A8)) [32, 2, 256].
- wq matmul: fp8 DoubleRow, stationary W-half, moving q8 -> psum = A8*(q.w),
  in two phases (all r-half-0 blocks, then all r-half-1) so the stationary
  loads only twice.
- exp: per block-pair either exact (scalar activation Exp -> fp8) or a
  "pair Schraudolph": u8 bits v1 = round(psum + B1), v2 = round(psum + B1+4);
  the out-matmul sums the two phase-shifted approximations via two kv
  stationaries (cancels the exp2 linear-interp sawtooth).
- out matmul: fp8 DoubleRow, stationary kv8 variants, out^T [64, 512] per
  block; even/odd blocks pack into one [128, 512] psum tile.
- K-side: bf16 wk matmul (k_t chunks stationary); exp split scalar-exact /
  DVE bf16-pair-schraudolph; kv^T accumulation; PE transpose; fp8 convert.

Shapes hardcoded: B=4, N=4096, H=8, D=64, R=256.
"""

import math
import sys

sys.path.insert(0, "/opt/trn_rl_repo")

from contextlib import ExitStack

import ml_dtypes
import numpy as np

import concourse.bacc as bacc
import concourse.mybir as mybir
import concourse.tile as tile
from concourse import bass_utils

B, N, H, D, R = 4, 4096, 8, 64, 256
NCORES = 8
NH = (N // 2) * H          # 16384 q-rows per core
NBLK = NH // 512           # 32 out blocks of 512 rows
NPAIR = NBLK // 2          # 16 block pairs
KC = N // 128              # 32 k-chunks

FP32 = mybir.dt.float32
BF16 = mybir.dt.bfloat16
FP8 = mybir.dt.float8e4
U8 = mybir.dt.uint8
I16 = mybir.dt.int16
EXP = mybir.ActivationFunctionType.Exp
COPY = mybir.ActivationFunctionType.Copy
ADD = mybir.AluOpType.add
MULT = mybir.AluOpType.mult
DR = mybir.MatmulPerfMode.DoubleRow

A8 = 8.0 / math.log(2.0)            # fp8 bits per e-fold
SA8 = math.sqrt(A8)
C8P = -1.65                          # fp8 pair-schraudolph offset (round-nearest)
B8_1 = 56.0 + C8P
B8_2 = B8_1 + 4.0
WA8, WB8 = 0.555, 0.39244            # fp8 pair combination weights
A16 = 128.0 / math.log(2.0)          # bf16 bits per e-fold
B16P = 16256.0 - 6.9                 # bf16 pair offset
PAIR16_SCALE = 1.0 + 2.0 ** -0.5     # wk pair outputs (1+1/sqrt2)*exp
WK_BIAS = math.log(PAIR16_SCALE)     # scalar wk tiles match via exp bias
VSCALE = 0.25 / PAIR16_SCALE         # host folds into V
OSCALE = 0.25                        # kv8 tiles fold 1/OSCALE

# per-pair conv type: 'E' exact (scalar), 'R' pair-schraudolph
# (v1 = DVE convert from psum; v2 = v1 + 4 via uint8 accumulate-DMA)
PAIR_TYPE = ['E'] * 3 + ['R'] * 10 + ['E'] * 3   # R contiguous mid, E at ends
R_ENGINES = ['D']                    # engine cycle for R op1 units
WK_ASSIGN = ['S'] * 8                # wk conv tiles
OC_ENGINES = ['D', 'S'] * 8


def _build_program():
    nc = bacc.Bacc(
        "TRN2",
        target_bir_lowering=False,
        debug=False,
        enable_asserts=False,
        num_devices=NCORES,
    )
    qt = nc.dram_tensor("qt", [64, NH], FP8, kind="ExternalInput").ap()
    kt = nc.dram_tensor("kt", [64, N], FP8, kind="ExternalInput").ap()
    w8 = nc.dram_tensor("w8", [64, R], FP8, kind="ExternalInput").ap()
    v = nc.dram_tensor("v", [128, KC, D], BF16, kind="ExternalInput").ap()
    w = nc.dram_tensor("w", [64, R], BF16, kind="ExternalInput").ap()
    ident = nc.dram_tensor("ident", [128, 128], BF16, kind="ExternalInput").ap()
    o = nc.dram_tensor("o", [NPAIR, 128, 512], BF16, kind="ExternalOutput").ap()

    def eng(c):
        return {'S': nc.scalar, 'D': nc.vector, 'P': nc.gpsimd}[c]

    with tile.TileContext(nc) as tc, ExitStack() as ctx:
        consts = ctx.enter_context(tc.tile_pool(name="consts", bufs=1))
        id_sb = consts.tile([128, 128], BF16, tag="id")
        w_sb = consts.tile([64, R], BF16, tag="w")
        w8_sb = consts.tile([64, R], FP8, tag="w8")
        kt_sb = consts.tile([64, N], FP8, tag="kt")
        v_sb = consts.tile([128, KC, D], BF16, tag="v")
        kvt_sb = consts.tile([64, R], BF16, tag="kvt")
        # zero-padded out-matmul stationaries: lo fills out partitions 0:64
        # (even block), hi fills 64:128 (odd block, accumulated)
        kv8s = {}
        for nm in ("e_lo", "e_hi", "a_lo", "a_hi", "b_lo", "b_hi"):
            kv8s[nm] = consts.tile([128, 2, 128], FP8, tag=f"kv8{nm}",
                                   name=f"kv8{nm}")
            nc.vector.memset(kv8s[nm][:], 0.0)
        # wqe storage: [128, blk, rblock, 512] fp8 (v2 only written for R pairs)
        wqe1 = consts.tile([128, NBLK, 2, 512], FP8, tag="wqe1")
        wqe2 = consts.tile([128, NBLK, 2, 512], FP8, tag="wqe2")
        dummy = consts.tile([128, 8], FP32, tag="dummy")
        dummy2 = consts.tile([128, 8], FP32, tag="dummy2")
        wkbias = consts.tile([128, 1], FP32, tag="wkbias")
        nc.vector.memset(wkbias[:], WK_BIAS)
        four_sb = consts.tile([128, 1024], U8, tag="four")
        nc.vector.memset(four_sb[:], 4)

        qtpool = ctx.enter_context(tc.tile_pool(name="qtp", bufs=4))
        wkepool = ctx.enter_context(tc.tile_pool(name="wkep", bufs=8))
        i16pool = ctx.enter_context(tc.tile_pool(name="i16p", bufs=2))
        osbpool = ctx.enter_context(tc.tile_pool(name="osbp", bufs=3))
        # PSUM: mmps [128,2,512] fp32 x2 = 4 banks; ops [128,512] x2 = 2;
        # kvps 1; trps 1 -> 8 banks
        mmps = ctx.enter_context(tc.tile_pool(name="mmps", bufs=2, space="PSUM"))
        ops = ctx.enter_context(tc.tile_pool(name="ops", bufs=2, space="PSUM"))
        kvpsp = ctx.enter_context(tc.tile_pool(name="kvpsp", bufs=1, space="PSUM"))
        trpsp = ctx.enter_context(tc.tile_pool(name="trpsp", bufs=1, space="PSUM"))

        nc.vector.memset(dummy[:], 0.0)
        # ---- input DMAs on 3 rings ----
        # sync ring: kt, v (K-side feed), then v2 presets + output tiles
        # scalar ring: qt chunks 0-1;  gpsimd ring: w, ident, qt chunks 2-3
        for t in range(2):
            nc.sync.dma_start(
                kt_sb[:, 2048 * t : 2048 * (t + 1)],
                kt[:, 2048 * t : 2048 * (t + 1)],
            )
        for t in range(2):
            nc.sync.dma_start(
                v_sb[:, 16 * t : 16 * (t + 1), :], v[:, 16 * t : 16 * (t + 1), :]
            )
        qtc = []
        for t in range(4):
            qtt = qtpool.tile([64, 4096], FP8, tag="qt", name=f"qt_{t}")
            qtc.append(qtt)
        nc.scalar.dma_start(w8_sb[:], w8)
        nc.scalar.dma_start(qtc[0][:], qt[:, 0:4096])
        nc.scalar.dma_start(qtc[1][:], qt[:, 4096:8192])
        nc.scalar.dma_start(w_sb[:], w)
        nc.scalar.dma_start(id_sb[:], ident)
        nc.gpsimd.dma_start(qtc[2][:], qt[:, 8192:12288])
        nc.gpsimd.dma_start(qtc[3][:], qt[:, 12288:16384])
        # warm the scalar-engine exp table (after DMA issues, before convs)
        nc.scalar.activation(dummy2[:], dummy[:], EXP)
        # preset v2 slots of R pairs with 4s (bits offset for the pair trick)
        for p in range(NPAIR):
            if PAIR_TYPE[p] == 'R':
                for h in range(2):
                    nc.sync.dma_start(
                        wqe2[:, 2 * p : 2 * p + 2, h, :].bitcast(U8), four_sb[:]
                    )

        # ---- K-side: wk = exp(K @ W), tiles of 4 chunks in [128,2,512] ----
        wkes = []
        for t in range(8):
            wkps = mmps.tile([128, 2, 512], FP32, tag="mm", name=f"wkps{t}")
            for j in range(4):
                c = 4 * t + j
                nc.tensor.matmul(
                    wkps[:, j // 2, 256 * (j % 2) : 256 * (j % 2 + 1)],
                    kt_sb[:, 128 * c : 128 * (c + 1)],
                    w8_sb[:],
                )
            wke = wkepool.tile([128, 2, 512], BF16, tag="wke", name=f"wke{t}")
            if WK_ASSIGN[t] == 'S':
                nc.scalar.activation(wke[:], wkps[:], EXP, scale=1.0 / A8,
                                     bias=wkbias[:])
            else:
                i1 = i16pool.tile([128, 2, 512], I16, tag="i16a", name=f"i16a{t}")
                i2 = i16pool.tile([128, 2, 512], I16, tag="i16b", name=f"i16b{t}")
                nc.vector.tensor_scalar(i1[:], wkps[:], A16, B16P, MULT, ADD)
                nc.vector.tensor_scalar(i2[:], i1[:], -64, None, ADD)
                nc.vector.tensor_tensor(
                    wke[:], i1[:].bitcast(BF16), i2[:].bitcast(BF16), ADD
                )
            wkes.append(wke)

        # ---- Q-side phase h: wq psum = A8*(q.w) for r-half h ----
        r_rr = [0]

        def conv_pair(p, h, wqps):
            dst1 = wqe1[:, 2 * p : 2 * p + 2, h, :]
            if PAIR_TYPE[p] == 'E':
                nc.scalar.activation(dst1, wqps[:], EXP, scale=1.0 / A8)
            else:
                # split the convert across scalar and DVE halves (halves the
                # pipeline latency); v2 batches via accumulate-DMA later
                u1 = dst1.bitcast(U8)
                nc.scalar.activation(u1[:, :, 0:192], wqps[:, :, 0:192],
                                     COPY, bias=B8_1)
                nc.vector.tensor_scalar(u1[:, :, 192:512], wqps[:, :, 192:512],
                                        B8_1, None, ADD)
            if p in (4, 6, 8, 10, 12):
                lo = p - 1 if p > 4 else 3
                nc.gpsimd.dma_start(
                    wqe2[:, 2 * lo : 2 * p + 2, h, :].bitcast(U8),
                    wqe1[:, 2 * lo : 2 * p + 2, h, :].bitcast(U8),
                    accum_op=ADD,
                )

        def wq_mm(p, h):
            wqps = mmps.tile([128, 2, 512], FP32, tag="mm", name=f"wqps{h}_{p}")
            for j in range(2):
                blk = 2 * p + j
                ch = qtc[blk // 8]
                col = (blk % 8) * 512
                nc.tensor.matmul(
                    wqps[:, j, :],
                    w8_sb[:, 128 * h : 128 * (h + 1)],
                    ch[:, col : col + 512],
                )
            return wqps

        for p in range(NPAIR):
            wqps = wq_mm(p, 0)
            conv_pair(p, 0, wqps)

        # ---- kv^T accumulation over all 32 chunks ----
        kvps = kvpsp.tile([64, R], FP32, tag="kvps")
        for c in range(KC):
            nc.tensor.matmul(
                kvps[:],
                v_sb[:, c, :],
                wkes[c // 4][:, (c % 4) // 2, 256 * (c % 2) : 256 * (c % 2 + 1)],
                start=(c == 0),
                stop=(c == KC - 1),
            )
        nc.scalar.activation(kvt_sb[:], kvps[:], COPY)
        # transpose kv^T -> kv [256, 64] (bf16 psum), then fp8 converts
        trps = trpsp.tile([128, 2, D], BF16, tag="trps")
        for j in range(2):
            nc.tensor.matmul(
                trps[:, j, :],
                kvt_sb[:, 128 * j : 128 * (j + 1)],
                id_sb[0:64, 0:64],
                is_transpose=True,
            )
        # kv8 tiles hold 0.25*kv_true (fp8e4 max is 240; kv peaks ~±300);
        # the final output copy scales by 4.
        for nm, sc in (("e", 1.0), ("a", WA8), ("b", WB8)):
            for half, off in (("lo", 0), ("hi", 64)):
                nc.vector.tensor_scalar(
                    kv8s[f"{nm}_{half}"][:, :, off : off + 64],
                    trps[:],
                    sc,
                    None,
                    MULT,
                )

        # ---- Q-side phase 1 interleaved with out matmuls ----
        def block_mms(p, j):
            """(stationary, rhs) list for block 2p+j of pair p."""
            blk = 2 * p + j
            half = 'lo' if j == 0 else 'hi'
            if PAIR_TYPE[p] == 'E':
                return [(kv8s[f"e_{half}"], wqe1[:, blk, :, :])]
            return [(kv8s[f"a_{half}"], wqe1[:, blk, :, :]),
                    (kv8s[f"b_{half}"], wqe2[:, blk, :, :])]

        def finish_pair(p, opst):
            o_sb = osbpool.tile([128, 512], BF16, tag="osb", name=f"osb{p}")
            if OC_ENGINES[p] == 'S':
                nc.scalar.activation(o_sb[:], opst[:], COPY, scale=4.0)
            else:
                nc.vector.tensor_scalar(o_sb[:], opst[:], 4.0, None, MULT)
            nc.sync.dma_start(o[p], o_sb[:])

        def emit_duo(pa, pb):
            """Out-matmuls for two pairs, interleaved so each stationary is
            used twice in a row (hides the DoubleRow weight loads)."""
            tiles = {pa: ops.tile([128, 512], FP32, tag="ops", name=f"ops{pa}"),
                     pb: ops.tile([128, 512], FP32, tag="ops", name=f"ops{pb}")}
            seqs = {p: block_mms(p, 0) + block_mms(p, 1) for p in (pa, pb)}
            nmax = max(len(seqs[pa]), len(seqs[pb]))
            for i in range(nmax):
                for p in (pa, pb):
                    if i < len(seqs[p]):
                        st, rhs = seqs[p][i]
                        nc.tensor.matmul(
                            tiles[p][:], st[:], rhs, perf_mode=DR,
                            start=(i == 0), stop=(i == len(seqs[p]) - 1),
                            skip_group_check=True,
                        )
            for p in (pa, pb):
                finish_pair(p, tiles[p])

        h1tiles = {}
        for g in range(8):  # groups of 2 pairs
            for p in range(2 * g, 2 * g + 2):
                h1tiles[p] = wq_mm(p, 1)
            for p in range(2 * g, 2 * g + 2):
                conv_pair(p, 1, h1tiles[p])
            if g >= 2:
                emit_duo(2 * (g - 2), 2 * (g - 2) + 1)
        emit_duo(12, 13)
        emit_duo(14, 15)

    nc.compile()
    return nc


_NC = None


def _get_nc():
    global _NC
    if _NC is None:
        _NC = _build_program()
    return _NC


def kernel(Q, K, V, W):
    nc = _get_nc()
    ident = np.eye(128, dtype=ml_dtypes.bfloat16)
    fp8np = mybir.dt.np(FP8)
    w_bf = np.ascontiguousarray(W).astype(ml_dtypes.bfloat16)
    w8_np = np.ascontiguousarray(W * SA8).astype(fp8np)
    in_maps = []
    for c in range(NCORES):
        b, half = c // 2, c % 2
        qs = Q[b, half * (N // 2) : (half + 1) * (N // 2)].reshape(NH, D)
        qt_np = np.ascontiguousarray(qs.T * SA8).astype(fp8np)
        kt_np = np.ascontiguousarray(K[b].T * SA8).astype(fp8np)
        v_np = np.ascontiguousarray(
            (V[b] * VSCALE).reshape(KC, 128, D).transpose(1, 0, 2)
        ).astype(ml_dtypes.bfloat16)
        in_maps.append({
            "qt": qt_np,
            "kt": kt_np,
            "v": v_np,
            "w": w_bf,
            "w8": w8_np,
            "ident": ident,
        })
    global _LAST_IN_MAPS
    _LAST_IN_MAPS = in_maps
    res = bass_utils.run_bass_kernel_spmd(nc, in_maps, core_ids=list(range(NCORES)))
    out = np.empty((B, N, H, D), np.float32)
    out_t = np.empty((D, NH), np.float32)
    for c in range(NCORES):
        b, half = c // 2, c % 2
        ob = res.results[c]["o"].astype(np.float32)   # [16, 128, 512]
        for p in range(NPAIR):
            out_t[:, 1024 * p : 1024 * p + 512] = ob[p, 0:64]
            out_t[:, 1024 * p + 512 : 1024 * (p + 1)] = ob[p, 64:128]
        out[b, half * (N // 2) : (half + 1) * (N // 2)] = out_t.T.reshape(
            N // 2, H, D
        )
    return out


# revision 31
# speedup vs baseline: 1.2355x; 1.0067x over previous
"""Bass/Tile kernel for KernelAttention (linear attention with exp random features).

Per batch b:  wk = exp(K @ W); kv = wk.T @ V; wq = exp(Q @ W); out = wq @ kv.

Sharding: 8 cores = 4 batches x 2 n-halves of Q. K-side computed redundantly
per core pair.

Per-core design:
- Host pre-transposes/converts inputs: q8 = fp8(Q^T * sqrt(A8)) in DoubleRow
  layout [32, 2, 16384]; k_t bf16 [64, 4096]; v bf16 [128, 32, 64] (scaled);
  w bf16; w8 = fp8(W * sqrt(A8)) [32, 2, 256].
- wq matmul: fp8 DoubleRow, stationary W-half, moving q8 -> psum = A8*(q.w),
  in two phases (all r-half-0 blocks, then all r-half-1) so the stationary
  loads only twice.
- exp: per block-pair either exact (scalar activation Exp -> fp8) or a
  "pair Schraudolph": u8 bits v1 = round(psum + B1), v2 = round(psum + B1+4);
  the out-matmul sums the two phase-shifted approximations via two kv
  stationaries (cancels the exp2 linear-interp sawtooth).
- out matmul: fp8 DoubleRow, stationary kv8 variants, out^T [64, 512] per
  block; even/odd blocks pack into one [128, 512] psum tile.
- K-side: bf16 wk matmul (k_t chunks stationary); exp split scalar-exact /
  DVE bf16-pair-schraudolph; kv^T accumulation; PE transpose; fp8 convert.

Shapes hardcoded: B=4, N=4096, H=8, D=64, R=256.
"""

import math
import sys

sys.path.insert(0, "/opt/trn_rl_repo")

from contextlib import ExitStack

import ml_dtypes
import numpy as np

import concourse.bacc as bacc
import concourse.mybir as mybir
import concourse.tile as tile
from concourse import bass_utils

B, N, H, D, R = 4, 4096, 8, 64, 256
NCORES = 8
NH = (N // 2) * H          # 16384 q-rows per core
NBLK = NH // 512           # 32 out blocks of 512 rows
NPAIR = NBLK // 2          # 16 block pairs
KC = N // 128              # 32 k-chunks

FP32 = mybir.dt.float32
BF16 = mybir.dt.bfloat16
FP8 = mybir.dt.float8e4
U8 = mybir.dt.uint8
I16 = mybir.dt.int16
EXP = mybir.ActivationFunctionType.Exp
COPY = mybir.ActivationFunctionType.Copy
ADD = mybir.AluOpType.add
MULT = mybir.AluOpType.mult
DR = mybir.MatmulPerfMode.DoubleRow

A8 = 8.0 / math.log(2.0)            # fp8 bits per e-fold
SA8 = math.sqrt(A8)
C8P = -1.65                          # fp8 pair-schraudolph offset (round-nearest)
B8_1 = 56.0 + C8P
B8_2 = B8_1 + 4.0
WA8, WB8 = 0.555, 0.39244            # fp8 pair combination weights
A16 = 128.0 / math.log(2.0)          # bf16 bits per e-fold
B16P = 16256.0 - 6.9                 # bf16 pair offset
PAIR16_SCALE = 1.0 + 2.0 ** -0.5     # wk pair outputs (1+1/sqrt2)*exp
WK_BIAS = math.log(PAIR16_SCALE)     # scalar wk tiles match via exp bias
VSCALE = 0.25 / PAIR16_SCALE         # host folds into V
OSCALE = 0.25                        # kv8 tiles fold 1/OSCALE

# per-pair conv type: 'E' exact (scalar), 'R' pair-schraudolph
# (v1 = DVE convert from psum; v2 = v1 + 4 via uint8 accumulate-DMA)
PAIR_TYPE = ['E'] * 3 + ['R'] * 10 + ['E'] * 3   # R contiguous mid, E at ends
R_ENGINES = ['D']                    # engine cycle for R op1 units
WK_ASSIGN = ['S', 'D', 'D', 'D', 'D', 'D', 'D', 'S']   # wk conv tiles
OC_ENGINES = ['D', 'S'] * 8


def _build_program():
    nc = bacc.Bacc(
        "TRN2",
        target_bir_lowering=False,
        debug=False,
        enable_asserts=False,
        num_devices=NCORES,
    )
    qt = nc.dram_tensor("qt", [64, NH], FP8, kind="ExternalInput").ap()
    kt = nc.dram_tensor("kt", [64, N], FP8, kind="ExternalInput").ap()
    w8 = nc.dram_tensor("w8", [64, R], FP8, kind="ExternalInput").ap()
    v = nc.dram_tensor("v", [128, KC, D], BF16, kind="ExternalInput").ap()
    w = nc.dram_tensor("w", [64, R], BF16, kind="ExternalInput").ap()
    ident = nc.dram_tensor("ident", [128, 128], BF16, kind="ExternalInput").ap()
    o = nc.dram_tensor("o", [NPAIR, 128, 512], BF16, kind="ExternalOutput").ap()

    def eng(c):
        return {'S': nc.scalar, 'D': nc.vector, 'P': nc.gpsimd}[c]

    with tile.TileContext(nc) as tc, ExitStack() as ctx:
        consts = ctx.enter_context(tc.tile_pool(name="consts", bufs=1))
        id_sb = consts.tile([128, 128], BF16, tag="id")
        w_sb = consts.tile([64, R], BF16, tag="w")
        w8_sb = consts.tile([64, R], FP8, tag="w8")
        kt_sb = consts.tile([64, N], FP8, tag="kt")
        v_sb = consts.tile([128, KC, D], BF16, tag="v")
        kvt_sb = consts.tile([64, R], BF16, tag="kvt")
        # zero-padded out-matmul stationaries: lo fills out partitions 0:64
        # (even block), hi fills 64:128 (odd block, accumulated)
        kv8s = {}
        for nm in ("e_lo", "e_hi", "a_lo", "a_hi", "b_lo", "b_hi"):
            kv8s[nm] = consts.tile([128, 2, 128], FP8, tag=f"kv8{nm}",
                                   name=f"kv8{nm}")
            nc.vector.memset(kv8s[nm][:], 0.0)
        # wqe storage: [128, blk, rblock, 512] fp8 (v2 only written for R pairs)
        wqe1 = consts.tile([128, NBLK, 2, 512], FP8, tag="wqe1")
        wqe2 = consts.tile([128, NBLK, 2, 512], FP8, tag="wqe2")
        dummy = consts.tile([128, 8], FP32, tag="dummy")
        dummy2 = consts.tile([128, 8], FP32, tag="dummy2")
        wkbias = consts.tile([128, 1], FP32, tag="wkbias")
        nc.vector.memset(wkbias[:], WK_BIAS)
        four_sb = consts.tile([128, 1024], U8, tag="four")
        nc.vector.memset(four_sb[:], 4)

        qtpool = ctx.enter_context(tc.tile_pool(name="qtp", bufs=4))
        wkepool = ctx.enter_context(tc.tile_pool(name="wkep", bufs=8))
        i16pool = ctx.enter_context(tc.tile_pool(name="i16p", bufs=2))
        osbpool = ctx.enter_context(tc.tile_pool(name="osbp", bufs=3))
        # PSUM: mmps [128,2,512] fp32 x2 = 4 banks; ops [128,512] x2 = 2;
        # kvps 1; trps 1 -> 8 banks
        mmps = ctx.enter_context(tc.tile_pool(name="mmps", bufs=2, space="PSUM"))
        ops = ctx.enter_context(tc.tile_pool(name="ops", bufs=2, space="PSUM"))
        kvpsp = ctx.enter_context(tc.tile_pool(name="kvpsp", bufs=1, space="PSUM"))
        trpsp = ctx.enter_context(tc.tile_pool(name="trpsp", bufs=1, space="PSUM"))

        nc.vector.memset(dummy[:], 0.0)
        # ---- input DMAs on 3 rings ----
        # sync ring: kt, v (K-side feed), then v2 presets + output tiles
        # scalar ring: qt chunks 0-1;  gpsimd ring: w, ident, qt chunks 2-3
        for t in range(2):
            nc.sync.dma_start(
                kt_sb[:, 2048 * t : 2048 * (t + 1)],
                kt[:, 2048 * t : 2048 * (t + 1)],
            )
        for t in range(2):
            nc.sync.dma_start(
                v_sb[:, 16 * t : 16 * (t + 1), :], v[:, 16 * t : 16 * (t + 1), :]
            )
        qtc = []
        for t in range(4):
            qtt = qtpool.tile([64, 4096], FP8, tag="qt", name=f"qt_{t}")
            qtc.append(qtt)
        nc.scalar.dma_start(w8_sb[:], w8)
        nc.scalar.dma_start(qtc[0][:], qt[:, 0:4096])
        nc.scalar.dma_start(qtc[1][:], qt[:, 4096:8192])
        nc.scalar.dma_start(w_sb[:], w)
        nc.scalar.dma_start(id_sb[:], ident)
        nc.gpsimd.dma_start(qtc[2][:], qt[:, 8192:12288])
        nc.gpsimd.dma_start(qtc[3][:], qt[:, 12288:16384])
        # warm the scalar-engine exp table (after DMA issues, before convs)
        nc.scalar.activation(dummy2[:], dummy[:], EXP)
        # preset v2 slots of R pairs with 4s (bits offset for the pair trick)
        for p in range(NPAIR):
            if PAIR_TYPE[p] == 'R':
                for h in range(2):
                    nc.sync.dma_start(
                        wqe2[:, 2 * p : 2 * p + 2, h, :].bitcast(U8), four_sb[:]
                    )

        # ---- K-side: wk = exp(K @ W), tiles of 4 chunks in [128,2,512] ----
        wkes = []
        for t in range(8):
            wkps = mmps.tile([128, 2, 512], FP32, tag="mm", name=f"wkps{t}")
            for j in range(4):
                c = 4 * t + j
                nc.tensor.matmul(
                    wkps[:, j // 2, 256 * (j % 2) : 256 * (j % 2 + 1)],
                    kt_sb[:, 128 * c : 128 * (c + 1)],
                    w8_sb[:],
                )
            wke = wkepool.tile([128, 2, 512], BF16, tag="wke", name=f"wke{t}")
            if WK_ASSIGN[t] == 'S':
                nc.scalar.activation(wke[:], wkps[:], EXP, scale=1.0 / A8,
                                     bias=wkbias[:])
            else:
                i1 = i16pool.tile([128, 2, 512], I16, tag="i16a", name=f"i16a{t}")
                i2 = i16pool.tile([128, 2, 512], I16, tag="i16b", name=f"i16b{t}")
                nc.vector.tensor_scalar(i1[:], wkps[:], A16 / A8, B16P, MULT, ADD)
                nc.vector.tensor_scalar(i2[:], i1[:], -64, None, ADD)
                nc.vector.tensor_tensor(
                    wke[:], i1[:].bitcast(BF16), i2[:].bitcast(BF16), ADD
                )
            wkes.append(wke)

        # ---- Q-side phase h: wq psum = A8*(q.w) for r-half h ----
        r_rr = [0]

        def conv_pair(p, h, wqps):
            dst1 = wqe1[:, 2 * p : 2 * p + 2, h, :]
            if PAIR_TYPE[p] == 'E':
                nc.scalar.activation(dst1, wqps[:], EXP, scale=1.0 / A8)
            else:
                # split the convert across scalar and DVE halves (halves the
                # pipeline latency); v2 batches via accumulate-DMA later
                u1 = dst1.bitcast(U8)
                nc.scalar.activation(u1[:, :, 0:192], wqps[:, :, 0:192],
                                     COPY, bias=B8_1)
                nc.vector.tensor_scalar(u1[:, :, 192:512], wqps[:, :, 192:512],
                                        B8_1, None, ADD)
            if p in (4, 6, 8, 10, 12):
                lo = p - 1 if p > 4 else 3
                nc.gpsimd.dma_start(
                    wqe2[:, 2 * lo : 2 * p + 2, h, :].bitcast(U8),
                    wqe1[:, 2 * lo : 2 * p + 2, h, :].bitcast(U8),
                    accum_op=ADD,
                )

        def wq_mm(p, h):
            wqps = mmps.tile([128, 2, 512], FP32, tag="mm", name=f"wqps{h}_{p}")
            for j in range(2):
                blk = 2 * p + j
                ch = qtc[blk // 8]
                col = (blk % 8) * 512
                nc.tensor.matmul(
                    wqps[:, j, :],
                    w8_sb[:, 128 * h : 128 * (h + 1)],
                    ch[:, col : col + 512],
                )
            return wqps

        for p in range(NPAIR):
            wqps = wq_mm(p, 0)
            conv_pair(p, 0, wqps)

        # ---- kv^T accumulation over all 32 chunks ----
        kvps = kvpsp.tile([64, R], FP32, tag="kvps")
        for c in range(KC):
            nc.tensor.matmul(
                kvps[:],
                v_sb[:, c, :],
                wkes[c // 4][:, (c % 4) // 2, 256 * (c % 2) : 256 * (c % 2 + 1)],
                start=(c == 0),
                stop=(c == KC - 1),
            )
        nc.scalar.activation(kvt_sb[:], kvps[:], COPY)
        # transpose kv^T -> kv [256, 64] (bf16 psum), then fp8 converts
        trps = trpsp.tile([128, 2, D], BF16, tag="trps")
        for j in range(2):
            nc.tensor.matmul(
                trps[:, j, :],
                kvt_sb[:, 128 * j : 128 * (j + 1)],
                id_sb[0:64, 0:64],
                is_transpose=True,
            )
        # kv8 tiles hold 0.25*kv_true (fp8e4 max is 240; kv peaks ~±300);
        # the final output copy scales by 4.
        for nm, sc in (("e", 1.0), ("a", WA8), ("b", WB8)):
            for half, off in (("lo", 0), ("hi", 64)):
                nc.vector.tensor_scalar(
                    kv8s[f"{nm}_{half}"][:, :, off : off + 64],
                    trps[:],
                    sc,
                    None,
                    MULT,
                )

        # ---- Q-side phase 1 interleaved with out matmuls ----
        def block_mms(p, j):
            """(stationary, rhs) list for block 2p+j of pair p."""
            blk = 2 * p + j
            half = 'lo' if j == 0 else 'hi'
            if PAIR_TYPE[p] == 'E':
                return [(kv8s[f"e_{half}"], wqe1[:, blk, :, :])]
            return [(kv8s[f"a_{half}"], wqe1[:, blk, :, :]),
                    (kv8s[f"b_{half}"], wqe2[:, blk, :, :])]

        def finish_pair(p, opst):
            o_sb = osbpool.tile([128, 512], BF16, tag="osb", name=f"osb{p}")
            if OC_ENGINES[p] == 'S':
                nc.scalar.activation(o_sb[:], opst[:], COPY, scale=4.0)
            else:
                nc.vector.tensor_scalar(o_sb[:], opst[:], 4.0, None, MULT)
            nc.sync.dma_start(o[p], o_sb[:])

        def emit_duo(pa, pb):
            """Out-matmuls for two pairs, interleaved so each stationary is
            used twice in a row (hides the DoubleRow weight loads)."""
            tiles = {pa: ops.tile([128, 512], FP32, tag="ops", name=f"ops{pa}"),
                     pb: ops.tile([128, 512], FP32, tag="ops", name=f"ops{pb}")}
            seqs = {p: block_mms(p, 0) + block_mms(p, 1) for p in (pa, pb)}
            nmax = max(len(seqs[pa]), len(seqs[pb]))
            for i in range(nmax):
                for p in (pa, pb):
                    if i < len(seqs[p]):
                        st, rhs = seqs[p][i]
                        nc.tensor.matmul(
                            tiles[p][:], st[:], rhs, perf_mode=DR,
                            start=(i == 0), stop=(i == len(seqs[p]) - 1),
                            skip_group_check=True,
                        )
            for p in (pa, pb):
                finish_pair(p, tiles[p])

        h1tiles = {}
        for g in range(8):  # groups of 2 pairs
            for p in range(2 * g, 2 * g + 2):
                h1tiles[p] = wq_mm(p, 1)
            for p in range(2 * g, 2 * g + 2):
                conv_pair(p, 1, h1tiles[p])
            if g >= 2:
                emit_duo(2 * (g - 2), 2 * (g - 2) + 1)
        emit_duo(12, 13)
        emit_duo(14, 15)

    nc.compile()
    return nc


_NC = None


def _get_nc():
    global _NC
    if _NC is None:
        _NC = _build_program()
    return _NC


def kernel(Q, K, V, W):
    nc = _get_nc()
    ident = np.eye(128, dtype=ml_dtypes.bfloat16)
    fp8np = mybir.dt.np(FP8)
    w_bf = np.ascontiguousarray(W).astype(ml_dtypes.bfloat16)
    w8_np = np.ascontiguousarray(W * SA8).astype(fp8np)
    in_maps = []
    for c in range(NCORES):
        b, half = c // 2, c % 2
        qs = Q[b, half * (N // 2) : (half + 1) * (N // 2)].reshape(NH, D)
        qt_np = np.ascontiguousarray(qs.T * SA8).astype(fp8np)
        kt_np = np.ascontiguousarray(K[b].T * SA8).astype(fp8np)
        v_np = np.ascontiguousarray(
            (V[b] * VSCALE).reshape(KC, 128, D).transpose(1, 0, 2)
        ).astype(ml_dtypes.bfloat16)
        in_maps.append({
            "qt": qt_np,
            "kt": kt_np,
            "v": v_np,
            "w": w_bf,
            "w8": w8_np,
            "ident": ident,
        })
    global _LAST_IN_MAPS
    _LAST_IN_MAPS = in_maps
    res = bass_utils.run_bass_kernel_spmd(nc, in_maps, core_ids=list(range(NCORES)))
    out = np.empty((B, N, H, D), np.float32)
    out_t = np.empty((D, NH), np.float32)
    for c in range(NCORES):
        b, half = c // 2, c % 2
        ob = res.results[c]["o"].astype(np.float32)   # [16, 128, 512]
        for p in range(NPAIR):
            out_t[:, 1024 * p : 1024 * p + 512] = ob[p, 0:64]
            out_t[:, 1024 * p + 512 : 1024 * (p + 1)] = ob[p, 64:128]
        out[b, half * (N // 2) : (half + 1) * (N // 2)] = out_t.T.reshape(
            N // 2, H, D
        )
    return out
